# revision 17
# baseline (speedup 1.0000x reference)
"""GCN (2-layer GraphConv x 2 graphs) on 8 Trainium2 NeuronCores.

Sharding: 1D dst-node partition (6250 nodes/core). Each core processes the
edges whose dst lands in its slab. Inputs ship sharded: core c receives only
its x^T slab [128, 6250]; on device each core computes its slab of XW0 = x@W0
(layer-1 linearity lets the spmm run on the projected table), then the slabs
are AllGathered into a full [50000, 128] bf16 table. The per-edge gather
reads 256B table rows via gpsimd dma_gather. The segment-sum runs on the
tensor engine: per 128-edge tile a one-hot matrix M[e, slot] =
vals[e]*(dst_local[e]==slot) is built with one DVE tensor_scalar, and PSUM
accumulates out^T[feat, slot] += msg^T @ M per 128-node window. Layer 1's
PSUM window is therefore h1^T directly: relu(+b0) then @W1 produces the h2
slab, which is AllGathered (padded to 128 cols) for layer 2's gather.
Edges are host-sorted by (dst window, src half); src halves keep dma_gather's
int16 indices in range. Per-(window,half) tile counts are padded to the max
across cores so all 8 cores share one SPMD program.
"""
import sys

sys.path.insert(0, "/opt/trn_rl_repo")

import numpy as np
import ml_dtypes

N_NODES = 50000
N_EDGES = 600000
F_IN = 128
F_HID = 128
F_OUT = 64
C = 8
SLAB = N_NODES // C          # 6250
NWIN = (SLAB + 127) // 128   # 49
LAST_SLOTS = SLAB - 128 * (NWIN - 1)  # 106
HALF = N_NODES // 2          # 25000 (< 2^15 so int16 indices work)
import os as _os

CH_TILES = int(_os.environ.get("K_CH_TILES", "7"))    # tiles per dma_gather call (64-desc ring cap)
DMA_SCRATCH = int(_os.environ.get("K_DMA_SCRATCH", "16384"))  # SWDGE ring carveout
NQUEUES = int(_os.environ.get("K_NQUEUES", "4"))
GATHER_SHARED = int(_os.environ.get("K_GATHER_SHARED", "1"))  # gather straight from Shared AllGather buf
MBUILD_TS = int(_os.environ.get("K_MBUILD_TS", "1"))  # per-tile tensor_scalar M-build (DVE 4x mode)
DEBUG_GRAPHS = 2             # build only first N graphs
DEBUG_SKIP_AG = False        # skip allgathers (downstream reads garbage)
DEBUG_SKIP_L2 = False        # skip layer 2 spmm entirely

_bf16 = ml_dtypes.bfloat16


def _preprocess_graph(src, dst, vals):
    """Partition+sort edges; returns per-core streams and the shared plan."""
    src = np.asarray(src, np.int64)
    dst = np.asarray(dst, np.int64)
    vals = np.asarray(vals, np.float32)

    core = dst // SLAB
    dstl = dst % SLAB
    win = dstl // 128
    slot = dstl % 128
    half = (src >= HALF).astype(np.int64)
    idxr = (src - half * HALF).astype(np.int64)

    key = (core * 2 + half) * NWIN + win
    ngroups = C * 2 * NWIN
    counts = np.bincount(key, minlength=ngroups)
    tc = -(-counts // 128)  # ceil
    tmax = tc.reshape(C, 2, NWIN).max(axis=0)  # [2, NWIN]
    # every window needs at least one tile so its PSUM gets initialized
    tmax[0] = np.maximum(tmax[0], (tmax.sum(axis=0) == 0).astype(tmax.dtype))

    tile_off = np.zeros((2, NWIN), np.int64)
    tile_off[:, 1:] = np.cumsum(tmax, axis=1)[:, :-1]
    L = tmax.sum(axis=1) * 128  # edges per (core, half) stream, padded

    order = np.argsort(key, kind="stable")
    ksort = key[order]
    gstart = np.zeros(ngroups, np.int64)
    gstart[1:] = np.cumsum(counts)[:-1]
    cumcount = np.arange(len(src)) - gstart[ksort]

    csort = ksort // (2 * NWIN)
    hsort = (ksort // NWIN) % 2
    wsort = ksort % NWIN
    pos = tile_off[hsort, wsort] * 128 + cumcount

    pad_idx = np.int16(int(_os.environ.get("K_PAD_IDX", "0")))
    streams = []
    for h in (0, 1):
        idx_a = np.full((C, L[h]), pad_idx, np.int16)
        sl_a = np.zeros((C, L[h]), np.float32)
        vl_a = np.zeros((C, L[h]), np.float32)
        m = hsort == h
        idx_a[csort[m], pos[m]] = idxr[order][m].astype(np.int16)
        sl_a[csort[m], pos[m]] = slot[order][m].astype(np.float32)
        vl_a[csort[m], pos[m]] = vals[order][m]
        streams.append((idx_a, sl_a, vl_a))

    plan = {
        "tmax": tmax,          # [2, NWIN] tile counts (shared across cores)
        "tile_off": tile_off,  # [2, NWIN] stream tile offsets
        "L": L,                # [2] padded stream lengths (edges)
    }
    return streams, plan


def _wrap_idx16(a):
    # [L] int16 -> [16, L/16]: idx j at [j%16, j//16] (device replicates x8)
    L = a.shape[0]
    return a.reshape(L // 16, 16).T.copy()


def _wrap128(a):
    # [L] -> [128, L/128]: edge j at [j%128, j//128]
    L = a.shape[0]
    return a.reshape(L // 128, 128).T.copy()


def _chunks(total_tiles):
    out = []
    p = 0
    while p < total_tiles:
        n = min(CH_TILES, total_tiles - p)
        out.append((p, n))
        p += n
    return out


def _emit_streams(nc, pool, g, plan, tensors):
    """Load idx/slot/val streams into SBUF (idx replicated to 8 q7 groups)."""
    from concourse import mybir

    tmax = plan["tmax"]
    (ixs, sls, vls) = tensors
    idxp, slvp = pool
    stream_sb = []
    for h in (0, 1):
        total_tiles = int(tmax[h].sum())
        ix_s = idxp.tile([128, total_tiles * 8], mybir.dt.int16, tag=f"ixf{g}{h}")
        for k in range(8):
            eng = nc.sync if k % 2 == 0 else nc.scalar
            eng.dma_start(out=ix_s[16 * k:16 * (k + 1), :], in_=ixs[h][:, :])
        sl_s = slvp.tile([128, total_tiles], mybir.dt.bfloat16, tag=f"slf{g}{h}")
        nc.scalar.dma_start(out=sl_s[:], in_=sls[h][:, :])
        vl_s = slvp.tile([128, total_tiles], mybir.dt.bfloat16, tag=f"vlf{g}{h}")
        nc.scalar.dma_start(out=vl_s[:], in_=vls[h][:, :])
        stream_sb.append((ix_s, sl_s, vl_s))
    return stream_sb


def _emit_xw0(nc, pool, g, tensors):
    """Project this core's x^T slab: xw0_slab[n, f] = (x @ W0) rows."""
    from concourse import mybir

    xt_s, w0_s, xw0s_d = tensors
    sbuf, psB = pool
    for t in range(NWIN):
        rows = 128 if t < NWIN - 1 else LAST_SLOTS
        ps = psB.tile([128, F_HID], mybir.dt.float32, space="PSUM", tag="ps_xw0")
        nc.tensor.matmul(out=ps[:rows, :], lhsT=xt_s[:, t * 128:t * 128 + rows],
                         rhs=w0_s[:], start=True, stop=True)
        xw_sb = sbuf.tile([128, F_HID], mybir.dt.bfloat16, tag="xw_sb")
        nc.scalar.activation(out=xw_sb[:rows, :], in_=ps[:rows, :],
                             func=mybir.ActivationFunctionType.Copy)
        eng = nc.sync if t % 2 == 0 else nc.scalar
        eng.dma_start(out=xw0s_d[t * 128:t * 128 + rows, :], in_=xw_sb[:rows, :])


def _emit_spmm(nc, pool, g, plan, stream_sb, table_t, feat, layer, flush, qctr):
    """Gather+M-build chunks, then per-window matmul accumulation,
    calling flush(w, ps, slots) right after each window's matmuls."""
    from concourse import mybir

    tmax, tile_off = plan["tmax"], plan["tile_off"]
    sbuf, msgp, mp, iota_s, psA, psC = pool

    msg_chunks = [[], []]
    m_chunks = [[], []]
    for h in (0, 1):
        total_tiles = int(tmax[h].sum())
        ix_s, sl_s, vl_s = stream_sb[h]
        for (p0, nt) in _chunks(total_tiles):
            msg = msgp.tile([128, nt, feat], mybir.dt.bfloat16, tag=f"msg{h}")
            nc.gpsimd.dma_gather(
                out_ap=msg[:],
                in_ap=table_t[h * HALF:(h + 1) * HALF, :],
                idxs_ap=ix_s[:, p0 * 8:(p0 + nt) * 8],
                num_idxs=nt * 128,
                num_idxs_reg=nt * 128,
                elem_size=feat,
                queue_num=qctr[0] % NQUEUES,
            )
            qctr[0] += 1
            m_c = mp.tile([128, nt, 128], mybir.dt.bfloat16, tag=f"m{h}")
            nc.vector.tensor_tensor(
                out=m_c[:],
                in0=sl_s[:, p0:p0 + nt, None].to_broadcast([128, nt, 128]),
                in1=iota_s[:, None, :].to_broadcast([128, nt, 128]),
                op=mybir.AluOpType.is_equal,
            )
            nc.vector.tensor_tensor(
                out=m_c[:],
                in0=m_c[:],
                in1=vl_s[:, p0:p0 + nt, None].to_broadcast([128, nt, 128]),
                op=mybir.AluOpType.mult,
            )
            msg_chunks[h].append(msg)
            m_chunks[h].append(m_c)

    for w in range(NWIN):
        slots = 128 if w < NWIN - 1 else LAST_SLOTS
        ps = (psA if layer == 1 else psC).tile(
            [F_OUT if layer == 2 else 128, 128], mybir.dt.float32,
            space="PSUM", tag=f"ps_spmm{layer}")
        tiles = []
        for h in (0, 1):
            for k in range(int(tmax[h][w])):
                p = int(tile_off[h][w]) + k
                tiles.append((h, p // CH_TILES, p % CH_TILES))
        for i, (h, q, t) in enumerate(tiles):
            msg = msg_chunks[h][q]
            m_c = m_chunks[h][q]
            lhsT = msg[:, t, :] if layer == 1 else msg[:, t, 0:F_OUT]
            nc.tensor.matmul(
                out=ps[:, :slots],
                lhsT=lhsT,
                rhs=m_c[:, t, :slots],
                start=(i == 0),
                stop=(i == len(tiles) - 1),
            )
        flush(w, ps, slots)


def _build_and_run(graphs):
    """graphs: list of (xT_slabs, streams, plan, W0, b0, W1, b1) per graph."""
    from concourse import bacc, mybir, tile
    from concourse.bass_utils import run_bass_kernel_spmd

    nc = bacc.Bacc("TRN2", target_bir_lowering=False, debug=False, num_devices=C,
                   num_swdge_queues=NQUEUES, dynamic_dma_scratch_size=DMA_SCRATCH)

    tensors_all = []
    for g, (xT, streams, plan, W0, b0, W1, b1) in enumerate(graphs, start=1):
        xt_t = nc.declare_dram_parameter(f"gxt{g}", [F_IN, SLAB], mybir.dt.bfloat16, isOutput=False)
        ixs, sls, vls = [], [], []
        for h in (0, 1):
            Lh = int(plan["L"][h])
            ixs.append(nc.declare_dram_parameter(f"ix{g}{h}", [16, Lh // 16], mybir.dt.int16, isOutput=False))
            sls.append(nc.declare_dram_parameter(f"sl{g}{h}", [128, Lh // 128], mybir.dt.bfloat16, isOutput=False))
            vls.append(nc.declare_dram_parameter(f"vl{g}{h}", [128, Lh // 128], mybir.dt.bfloat16, isOutput=False))
        w0_t = nc.declare_dram_parameter(f"w{g}0", [F_IN, F_HID], mybir.dt.bfloat16, isOutput=False)
        w1_t = nc.declare_dram_parameter(f"w{g}1", [F_HID, F_OUT], mybir.dt.bfloat16, isOutput=False)
        b0_t = nc.declare_dram_parameter(f"b{g}0", [F_HID], mybir.dt.float32, isOutput=False)
        b1_t = nc.declare_dram_parameter(f"b{g}1", [F_OUT], mybir.dt.float32, isOutput=False)
        out_t = nc.declare_dram_parameter(f"o{g}", [F_OUT, SLAB], mybir.dt.bfloat16, isOutput=True)
        xw0s_d = nc.dram_tensor(f"xw0s{g}", [SLAB, F_HID], mybir.dt.bfloat16)
        xw0c_d = nc.dram_tensor(f"xw0c{g}", [N_NODES, F_HID], mybir.dt.bfloat16, addr_space="Shared")
        xw0f_d = (xw0c_d if GATHER_SHARED
                  else nc.dram_tensor(f"xw0f{g}", [N_NODES, F_HID], mybir.dt.bfloat16))
        h2s_d = nc.dram_tensor(f"h2s{g}", [SLAB, 128], mybir.dt.bfloat16)
        h2c_d = nc.dram_tensor(f"h2c{g}", [N_NODES, 128], mybir.dt.bfloat16, addr_space="Shared")
        h2f_d = h2c_d
        tensors_all.append((xt_t, ixs, sls, vls, w0_t, w1_t, b0_t, b1_t, out_t,
                            xw0s_d, xw0c_d, xw0f_d, h2s_d, h2f_d, h2c_d))
    iota_t = nc.declare_dram_parameter("iota", [128, 128], mybir.dt.bfloat16, isOutput=False)

    ngraphs = min(len(graphs), DEBUG_GRAPHS)
    with tile.TileContext(nc) as tc:
        with (
            tc.tile_pool(name="sbuf", bufs=3) as sbuf,
            tc.tile_pool(name="msgp", bufs=3) as msgp,
            tc.tile_pool(name="mp", bufs=3) as mp,
            tc.tile_pool(name="idxp", bufs=1) as idxp,
            tc.tile_pool(name="slvp", bufs=1) as slvp,
            tc.tile_pool(name="consts", bufs=1) as consts,
            tc.tile_pool(name="psA", bufs=2, space="PSUM") as psA,
            tc.tile_pool(name="psB", bufs=2, space="PSUM") as psB,
            tc.tile_pool(name="psC", bufs=2, space="PSUM") as psC,
        ):
            iota_s = consts.tile([128, 128], mybir.dt.bfloat16)
            nc.sync.dma_start(out=iota_s[:], in_=iota_t[:, :])
            consts_sb = []
            for g in range(1, ngraphs + 1):
                (xt_t, ixs, sls, vls, w0_t, w1_t, b0_t, b1_t, out_t,
                 xw0s_d, xw0c_d, xw0f_d, h2s_d, h2f_d, h2c_d) = tensors_all[g - 1]
                w0_s = consts.tile([F_IN, F_HID], mybir.dt.bfloat16, tag=f"w0_{g}")
                nc.sync.dma_start(out=w0_s[:], in_=w0_t[:, :])
                w1_s = consts.tile([F_HID, F_OUT], mybir.dt.bfloat16, tag=f"w1_{g}")
                nc.sync.dma_start(out=w1_s[:], in_=w1_t[:, :])
                b0_s = consts.tile([F_HID, 1], mybir.dt.float32, tag=f"b0_{g}")
                nc.sync.dma_start(out=b0_s[:, 0:1], in_=b0_t[:, None])
                b1_s = consts.tile([F_OUT, 1], mybir.dt.float32, tag=f"b1_{g}")
                nc.sync.dma_start(out=b1_s[:, 0:1], in_=b1_t[:, None])
                xt_s = consts.tile([F_IN, SLAB], mybir.dt.bfloat16, tag=f"xt_{g}")
                nc.sync.dma_start(out=xt_s[:], in_=xt_t[:, :])
                consts_sb.append((w0_s, w1_s, b0_s, b1_s, xt_s))

            qctr = [0]
            streams_sb = []
            # phase 0: project x slabs, allgather tables, load streams
            for g in range(1, ngraphs + 1):
                (xt_t, ixs, sls, vls, w0_t, w1_t, b0_t, b1_t, out_t,
                 xw0s_d, xw0c_d, xw0f_d, h2s_d, h2f_d, h2c_d) = tensors_all[g - 1]
                w0_s, w1_s, b0_s, b1_s, xt_s = consts_sb[g - 1]
                _emit_xw0(nc, (sbuf, psB), g, (xt_s, w0_s, xw0s_d))
                if not DEBUG_SKIP_AG:
                    nc.gpsimd.collective_compute(
                        "AllGather", mybir.AluOpType.bypass,
                        replica_groups=[list(range(C))],
                        ins=[xw0s_d[:]], outs=[xw0c_d[:]],
                    )
                    if not GATHER_SHARED:
                        nc.sync.dma_start(out=xw0f_d[:, :], in_=xw0c_d[:, :])
                streams_sb.append(_emit_streams(nc, (idxp, slvp), g,
                                                graphs[g - 1][2], (ixs, sls, vls)))

            # phase 1: layer-1 spmm per graph -> h2 slab -> allgather
            for g in range(1, ngraphs + 1):
                (xt_t, ixs, sls, vls, w0_t, w1_t, b0_t, b1_t, out_t,
                 xw0s_d, xw0c_d, xw0f_d, h2s_d, h2f_d, h2c_d) = tensors_all[g - 1]
                w0_s, w1_s, b0_s, b1_s, xt_s = consts_sb[g - 1]
                plan = graphs[g - 1][2]

                def flush_l1(w, ps, slots, w1_s=w1_s, b0_s=b0_s, h2s_d=h2s_d):
                    r1t = sbuf.tile([128, 128], mybir.dt.bfloat16, tag="r1t")
                    nc.scalar.activation(out=r1t[:, :slots], in_=ps[:, :slots],
                                         func=mybir.ActivationFunctionType.Relu,
                                         bias=b0_s[:, 0:1])
                    ps_out = psB.tile([128, F_OUT], mybir.dt.float32, space="PSUM", tag="ps_out")
                    nc.tensor.matmul(out=ps_out[:slots, :], lhsT=r1t[:, :slots],
                                     rhs=w1_s[:], start=True, stop=True)
                    h2sb = sbuf.tile([128, F_OUT], mybir.dt.bfloat16, tag="h2sb")
                    nc.scalar.activation(out=h2sb[:slots, :], in_=ps_out[:slots, :],
                                         func=mybir.ActivationFunctionType.Copy)
                    eng = nc.sync if w % 2 == 0 else nc.scalar
                    eng.dma_start(out=h2s_d[w * 128:w * 128 + slots, 0:F_OUT],
                                  in_=h2sb[:slots, :])

                _emit_spmm(nc, (sbuf, msgp, mp, iota_s, psA, psC), g, plan,
                           streams_sb[g - 1], xw0f_d, F_HID, 1, flush_l1, qctr)

                if not DEBUG_SKIP_AG:
                    nc.gpsimd.collective_compute(
                        "AllGather", mybir.AluOpType.bypass,
                        replica_groups=[list(range(C))],
                        ins=[h2s_d[:]], outs=[h2c_d[:]],
                    )

            # phase 2: layer-2 spmm per graph -> output
            if not DEBUG_SKIP_L2:
                for g in range(1, ngraphs + 1):
                    (xt_t, ixs, sls, vls, w0_t, w1_t, b0_t, b1_t, out_t,
                     xw0s_d, xw0c_d, xw0f_d, h2s_d, h2f_d, h2c_d) = tensors_all[g - 1]
                    w0_s, w1_s, b0_s, b1_s, xt_s = consts_sb[g - 1]
                    plan = graphs[g - 1][2]

                    def flush_l2(w, ps, slots, b1_s=b1_s, out_t=out_t):
                        o_sb = sbuf.tile([F_OUT, 128], mybir.dt.bfloat16, tag="o_sb")
                        nc.vector.tensor_scalar_add(
                            out=o_sb[:, :slots], in0=ps[:, :slots], scalar1=b1_s[:, 0:1])
                        eng = nc.sync if w % 2 == 0 else nc.scalar
                        eng.dma_start(out=out_t[:, w * 128:w * 128 + slots],
                                      in_=o_sb[:, :slots])

                    _emit_spmm(nc, (sbuf, msgp, mp, iota_s, psA, psC), g, plan,
                               streams_sb[g - 1],
                               h2f_d if not DEBUG_SKIP_AG else xw0f_d,
                               128, 2, flush_l2, qctr)

    nc.compile()

    # per-core input maps
    iota = np.tile(np.arange(128, dtype=np.float32), (128, 1))
    in_maps = []
    for c in range(C):
        m = {"iota": iota.astype(_bf16)}
        for g, (xT, streams, plan, W0, b0, W1, b1) in enumerate(graphs, start=1):
            m[f"gxt{g}"] = np.ascontiguousarray(xT[:, c * SLAB:(c + 1) * SLAB])
            for h in (0, 1):
                idx_a, sl_a, vl_a = streams[h]
                m[f"ix{g}{h}"] = _wrap_idx16(idx_a[c])
                m[f"sl{g}{h}"] = _wrap128(sl_a[c]).astype(_bf16)
                m[f"vl{g}{h}"] = _wrap128(vl_a[c]).astype(_bf16)
            m[f"w{g}0"] = W0.astype(_bf16)
            m[f"w{g}1"] = W1.astype(_bf16)
            m[f"b{g}0"] = b0.astype(np.float32)
            m[f"b{g}1"] = b1.astype(np.float32)
        in_maps.append(m)

    global _last_run
    _last_run = (nc, in_maps)
    res = run_bass_kernel_spmd(nc, in_maps, list(range(C)))
    return res.results


_last_run = None


def measure_exec_ns(n_iters=6):
    """Re-execute the last-built kernel with device-resident inputs; returns
    (t_min_ns, t_med_ns) wall time of one execution via the PJRT path."""
    import time
    from jax.sharding import Mesh, PartitionSpec, NamedSharding
    from jax.experimental.shard_map import shard_map
    import jax
    from concourse import mybir
    from concourse.bass2jax import _bass_exec_p, partition_id_tensor

    assert _last_run is not None
    nc, in_maps = _last_run
    partition_name = nc.partition_id_tensor.name if nc.partition_id_tensor else None

    in_names, out_names, out_avals, zero_shapes = [], [], [], []
    for alloc in nc.m.functions[0].allocations:
        if not isinstance(alloc, mybir.MemoryLocationSet):
            continue
        name = alloc.memorylocations[0].name
        if alloc.kind == "ExternalInput":
            if name != partition_name:
                in_names.append(name)
        elif alloc.kind == "ExternalOutput":
            out_names.append(name)
            shape = tuple(alloc.tensor_shape)
            dtype = mybir.dt.np(alloc.dtype)
            out_avals.append(jax.core.ShapedArray(shape, dtype))
            zero_shapes.append((shape, dtype))
    n_params = len(in_names)
    all_in_names = in_names + out_names
    if partition_name is not None:
        all_in_names = all_in_names + [partition_name]

    def _extra():
        return (partition_id_tensor(),) if partition_name is not None else ()

    def _body1(*args):
        return tuple(_bass_exec_p.bind(
            *args, *_extra(), out_avals=tuple(out_avals), in_names=tuple(all_in_names),
            out_names=tuple(out_names), lowering_input_output_aliases=(),
            sim_require_finite=True, sim_require_nnan=True, nc=nc))

    devices = jax.devices()[:C]
    mesh = Mesh(np.asarray(devices), ("core",))
    sh = NamedSharding(mesh, PartitionSpec("core"))

    concat_in = [np.concatenate([np.asarray(in_maps[c][nm]) for c in range(C)], axis=0)
                 for nm in in_names]
    dev_in = [jax.device_put(a, sh) for a in concat_in]

    def make(fn, nz):
        specs = (PartitionSpec("core"),) * (n_params + nz * len(out_avals))
        outs = (PartitionSpec("core"),) * (nz * len(out_avals))
        donate = tuple(range(n_params, n_params + nz * len(out_avals)))
        return jax.jit(shard_map(fn, mesh=mesh, in_specs=specs, out_specs=outs,
                                 check_rep=False),
                       donate_argnums=donate, keep_unused=True)

    f1 = make(_body1, 1)

    def zeros():
        return [jax.device_put(np.zeros((C * s[0], *s[1:]), d), sh)
                for s, d in zero_shapes]

    t1 = []
    for _ in range(n_iters):
        z = zeros()
        jax.block_until_ready(z)
        t0 = time.perf_counter()
        o = f1(*dev_in, *z)
        jax.block_until_ready(o)
        t1.append(time.perf_counter() - t0)
    return min(t1) * 1e9, sorted(t1)[len(t1) // 2] * 1e9


def kernel(x1, src1, dst1, vals1, x2, src2, dst2, vals2,
           W1_0, b1_0, W1_1, b1_1, W2_0, b2_0, W2_1, b2_1):
    graphs = []
    for (x, src, dst, vals, W0, b0, W1, b1) in (
        (x1, src1, dst1, vals1, W1_0, b1_0, W1_1, b1_1),
        (x2, src2, dst2, vals2, W2_0, b2_0, W2_1, b2_1),
    ):
        streams, plan = _preprocess_graph(src, dst, vals)
        xT = np.ascontiguousarray(np.asarray(x, np.float32).T).astype(_bf16)
        graphs.append((xT, streams, plan,
                       np.asarray(W0, np.float32), np.asarray(b0, np.float32),
                       np.asarray(W1, np.float32), np.asarray(b1, np.float32)))

    results = _build_and_run(graphs)

    out = np.zeros((2, N_NODES, F_OUT), np.float32)
    for g in (1, 2):
        for c in range(C):
            out[g - 1, c * SLAB:(c + 1) * SLAB, :] = \
                results[c][f"o{g}"].astype(np.float32).T
    return out
